# revision 27
# baseline (speedup 1.0000x reference)
"""Multi-head dot-product attention (RoPE, causal) on 8 NeuronCores.

Sharding: data-parallel over batch (2) x tensor-parallel over heads (16 -> 4
per core). Each core projects q/k/v for its 4 heads, runs causal attention,
and computes a partial output projection; the host sums the 4 partials per
batch element.

Device layout notes:
- All matmul operands are float32r (full-rate on the PE for moving dims
  >= 256; matmuls are emitted at width >= 256).
- Inputs are fed pre-transposed ([E, S]) so projections contract E on
  partitions; q/k are produced transposed per head ([D, S]).
- Scores are computed transposed (ST[s, t]) so the A@V contraction needs no
  on-chip transposes; softmax skips max-subtraction (scores are O(1) by
  construction).
- Attention loops t-blocks outer / heads inner; the previous t-block's
  output projection is interleaved between a head's QK and AV matmuls so
  the PE never waits on the exp (Activation engine) pipeline.
- Diagonal score sub-blocks are emitted at narrowed moving widths
  (512/384/256/256) with a single shifted causal mask table; the masked
  prefix of each narrowed e-tile is never read.
- The softmax denominator is accumulated on the Pool engine (esum) and
  reduced across partitions with ONE all-ones matmul per (h, t-block);
  1/den uses the fast DVE reciprocal approximation.
- RoPE uses a de-interleaved head dim (even dims | odd dims), folded into a
  host-side permutation of Wq/Wk columns; scores are permutation-invariant.
"""

import numpy as np
import ml_dtypes

_BF16 = ml_dtypes.bfloat16

B, S, E, N, D = 2, 2048, 2048, 16, 128
HL = 4           # local heads per core (8 cores = 2 batch x 4 head groups)
ND = HL * D      # 512
NT = S // 128    # 16 row tiles
NB = S // 512    # 4 row blocks
NE = E // 128    # 16 contraction tiles
MASK_VALUE = float(-0.7 * np.finfo(np.float32).max)

# diagonal sub-block emission: v -> (column offset, width) within the t-block
DIAG_OFF = (0, 128, 256, 256)
DIAG_W = (512, 384, 256, 256)
# mask table column slices ([128, 640] table, M[s, c] = MASK iff c-128 < s)
DIAG_MSK = ((128, 640), (128, 512), (128, 384), (0, 256))

_NC_CACHE = {}


def _build_module():
    import concourse.bass as bass
    import concourse.mybir as mybir
    import concourse.tile as tile
    from concourse import bacc

    f32 = mybir.dt.float32
    f32r = mybir.dt.float32r
    bf16 = mybir.dt.bfloat16
    Exp = mybir.ActivationFunctionType.Exp

    nc = bacc.Bacc("TRN2", target_bir_lowering=False, debug=False, num_devices=8)

    xq_d = nc.dram_tensor("xq_t", [NE // 2, NB, 128, 2, 512], bf16, kind="ExternalInput").ap()
    xkv_d = nc.dram_tensor("xkv_t", [NE // 2, NB, 128, 2, 512], bf16, kind="ExternalInput").ap()
    wq_d = nc.dram_tensor("wq", [E, ND], bf16, kind="ExternalInput").ap()
    wk_d = nc.dram_tensor("wk", [E, ND], bf16, kind="ExternalInput").ap()
    wv_d = nc.dram_tensor("wv", [E, ND], bf16, kind="ExternalInput").ap()
    wo_d = nc.dram_tensor("wo", [ND, E], bf16, kind="ExternalInput").ap()
    csd_d = nc.dram_tensor("csd", [128, S], bf16, kind="ExternalInput").ap()
    sns_d = nc.dram_tensor("sns", [128, S], bf16, kind="ExternalInput").ap()
    ones_d = nc.dram_tensor("ones", [128, 128], bf16, kind="ExternalInput").ap()
    msk_d = nc.dram_tensor("msk", [128, 640], f32, kind="ExternalInput").ap()
    out_d = nc.dram_tensor("out", [S, E], f32, kind="ExternalOutput").ap()

    def load_w_tiles(pool, dram, tag, ets, engine=None):
        """Per-et weight tiles: [128, 512] bf16 slabs of [E, ND]."""
        ws = []
        for et in ets:
            w = pool.tile([128, ND], bf16, tag=f"{tag}{et}", name=f"{tag}{et}")
            (engine or nc.gpsimd).dma_start(
                w[:], dram[bass.ds(128 * et, 128), :])
            ws.append(w)
        return ws

    with tile.TileContext(nc) as tc:
        with tc.tile_pool(name="qkp", bufs=1) as qk_pool, \
             tc.tile_pool(name="vp", bufs=1) as v_pool, \
             tc.tile_pool(name="cst", bufs=1) as cpool:
            qT = [qk_pool.tile([128, S], bf16, tag=f"qT{h}", name=f"qT{h}")
                  for h in range(HL)]
            kT = [qk_pool.tile([128, S], bf16, tag=f"kT{h}", name=f"kT{h}")
                  for h in range(HL)]
            vG = [v_pool.tile([128, 4 * ND], bf16, tag=f"vG{g}",
                              name=f"vG{g}") for g in range(4)]
            ones = cpool.tile([128, 128], bf16, tag="ones")
            msk = cpool.tile([128, 640], f32, tag="msk")

            # ---- fused QKV projection (RoPE tables live only here) ----
            with nc.named_scope("proj"), \
                 tc.tile_pool(name="tables", bufs=1) as tpool, \
                 tc.tile_pool(name="wqp", bufs=1) as wq_pool, \
                 tc.tile_pool(name="wkp", bufs=1) as wk_pool, \
                 tc.tile_pool(name="wvp", bufs=1) as wv_pool, \
                 tc.tile_pool(name="xp", bufs=16) as xpool, \
                 tc.tile_pool(name="qps", bufs=2, space="PSUM") as qps_pool, \
                 tc.tile_pool(name="rope", bufs=2) as rope_pool:
                csd = tpool.tile([128, S], bf16, tag="csd")
                sns = tpool.tile([128, S], bf16, tag="sns")

                def rope(dst, src_ps, tb):
                    tbs = bass.ts(tb, 512)
                    tmp = rope_pool.tile([128, 512], f32, tag="tmp",
                                         name="tmp")
                    nc.vector.tensor_mul(tmp[0:64, :], src_ps[64:128, :],
                                         sns[0:64, tbs])
                    nc.vector.tensor_mul(tmp[64:128, :], src_ps[0:64, :],
                                         sns[64:128, tbs])
                    nc.vector.tensor_mul(dst[:, tbs], src_ps[:],
                                         csd[:, tbs])
                    nc.vector.tensor_add(dst[:, tbs], dst[:, tbs], tmp[:])

                wq = load_w_tiles(wq_pool, wq_d, "wq", range(NE))
                wk = load_w_tiles(wk_pool, wk_d, "wk", range(NE))
                wv = load_w_tiles(wv_pool, wv_d, "wv", range(4))

                def x_tile(dram, engine, tag, ep, tb, split=False):
                    x = xpool.tile([128, 2, 512], bf16, tag=tag,
                                   name=tag)
                    srcv = dram[ep, tb]
                    if split:
                        engine.dma_start(x[:, 0], srcv[:, 0])
                        engine.dma_start(x[:, 1], srcv[:, 1])
                    else:
                        engine.dma_start(x[:], srcv)
                    return x

                for tb in range(NB):
                    # xq: 8 tiles across 2 rotating tags (bufs=10 -> deep)
                    qeng = ((nc.sync, nc.scalar) if tb == 0 else
                            (nc.sync, nc.scalar, nc.gpsimd))
                    xq = [x_tile(xq_d, qeng[ep % len(qeng)], "xq", ep, tb,
                                 split=(tb == 0 and ep == 0))
                          for ep in range(NE // 2)]
                    keng = ((nc.scalar, nc.sync) if tb == 0 else
                            (nc.scalar, nc.gpsimd, nc.sync))
                    xkv = [x_tile(xkv_d, keng[ep % len(keng)],
                                  "xkv", ep, tb)
                           for ep in range(NE // 2)]
                    if tb == 0:
                        nc.sync.dma_start(csd[:], csd_d[:])
                        nc.sync.dma_start(sns[:], sns_d[:])
                        nc.scalar.dma_start(ones[:], ones_d[:])
                        nc.scalar.dma_start(msk[:], msk_d[:])
                        wv += load_w_tiles(wv_pool, wv_d, "wv",
                                           range(4, NE), engine=nc.sync)
                    # -- Q subloop --
                    qps = [qps_pool.tile([128, 512], f32, tag=f"q{h}",
                                         name=f"qps{h}") for h in range(HL)]
                    for et in range(NE):
                        for h in range(HL):
                            nc.tensor.matmul(
                                qps[h][:], wq[et][:, bass.ts(h, 128)],
                                xq[et // 2][:, et % 2], start=(et == 0),
                                stop=(et == NE - 1))
                    for h in range(HL):
                        rope(qT[h], qps[h][:], tb)
                    # -- K subloop --
                    kps = [qps_pool.tile([128, 512], f32, tag=f"q{h}",
                                         name=f"kps{h}") for h in range(HL)]
                    for et in range(NE):
                        for h in range(HL):
                            nc.tensor.matmul(
                                kps[h][:], wk[et][:, bass.ts(h, 128)],
                                xkv[et // 2][:, et % 2], start=(et == 0),
                                stop=(et == NE - 1))
                    for h in range(HL):
                        rope(kT[h], kps[h][:], tb)
                    # -- V subloop --
                    for sv in range(4):
                        vps = qps_pool.tile([128, ND], f32, tag=f"q{sv}",
                                            name=f"vps{sv}")
                        for et in range(NE):
                            nc.tensor.matmul(
                                vps[:], xkv[et // 2][:, et % 2, bass.ts(sv, 128)],
                                wv[et][:], start=(et == 0),
                                stop=(et == NE - 1))
                        nc.scalar.copy(vG[tb][:, bass.ts(sv, 512)], vps[:])

            # ---- Attention + output projection (interleaved) ----
            with nc.named_scope("attn"), \
                 tc.tile_pool(name="wop", bufs=1) as wo_pool, \
                 tc.tile_pool(name="et", bufs=1) as et_pool, \
                 tc.tile_pool(name="uTp", bufs=2) as ut_pool, \
                 tc.tile_pool(name="esp", bufs=12) as es_pool, \
                 tc.tile_pool(name="rcp", bufs=2) as rcp_pool, \
                 tc.tile_pool(name="ob", bufs=3) as ob_pool, \
                 tc.tile_pool(name="sps", bufs=2, space="PSUM") as sps_pool, \
                 tc.tile_pool(name="ups", bufs=1, space="PSUM") as ups_pool, \
                 tc.tile_pool(name="dps", bufs=1, space="PSUM") as dps_pool, \
                 tc.tile_pool(name="ops", bufs=2, space="PSUM") as ops_pool:
                wo = []
                for h in range(HL):
                    w = wo_pool.tile([128, E], bf16, tag=f"wo{h}",
                                     name=f"wo{h}")
                    nc.sync.dma_start(w[:], wo_d[bass.ts(h, 128), :])
                    wo.append(w)
                eG = [et_pool.tile([128, 2048], bf16, tag=f"eG{g}",
                                   name=f"eG{g}") for g in range(4)]

                def e_ap(si, lo=0, hi=512):
                    base = 512 * (si % 4)
                    return eG[si // 4][:, base + lo:base + hi]

                def out_group(tt, uTt, final=False):
                    """Output projection for one 128-row t-tile."""
                    ttl = tt % 4
                    for ec in range(4):
                        op = ops_pool.tile([128, 512], f32, tag="op",
                                           name="op")
                        for hh in range(HL):
                            nc.tensor.matmul(
                                op[:], uTt[hh][:, bass.ts(ttl, 128)],
                                wo[hh][:, bass.ts(ec, 512)],
                                start=(hh == 0), stop=(hh == HL - 1))
                        ob = ob_pool.tile([128, 512], f32, tag="ob",
                                          name="ob")
                        nc.vector.tensor_copy(out=ob[:], in_=op[:])
                        eng = nc.scalar if (final and ec % 2) else nc.sync
                        eng.dma_start(
                            out_d[bass.ds(128 * tt, 128),
                                  bass.ds(512 * ec, 512)], ob[:])

                uT_prev = None
                prev_tb = None
                for tb in range(NB):
                    tbs = bass.ts(tb, 512)
                    nsi = 4 * (tb + 1)
                    uT_cur = []
                    for h in range(HL):
                        # -- QK: full-width si pairs below the diagonal --
                        for j in range(2 * tb):
                            sp = sps_pool.tile([128, 2, 512], f32,
                                               tag="sp", name="sp")
                            for p2 in range(2):
                                si = 2 * j + p2
                                nc.tensor.matmul(
                                    sp[:, p2], kT[h][:, bass.ts(si, 128)],
                                    qT[h][:, tbs], start=True, stop=True)
                            nc.scalar.activation(
                                e_ap(2 * j, 0, 1024),
                                sp[:].rearrange("p a b -> p (a b)"), Exp)
                        # -- QK: narrowed diagonal sub-blocks --
                        spd = [sps_pool.tile([128, 2, 512], f32, tag="sp",
                                             name="sp") for _ in range(2)]
                        for v in range(4):
                            si = 4 * tb + v
                            off, w = DIAG_OFF[v], DIAG_W[v]
                            sl = spd[v // 2][:, v % 2, off:512]
                            nc.tensor.matmul(
                                sl, kT[h][:, bass.ts(si, 128)],
                                qT[h][:, 512 * tb + off:512 * (tb + 1)],
                                start=True, stop=True)
                            m0, m1 = DIAG_MSK[v]
                            nc.vector.tensor_add(sl, sl, msk[:, m0:m1])
                            nc.scalar.activation(e_ap(si, off), sl, Exp)
                        # -- Pool pair/quad sums of e (non-diag) --
                        quads = []
                        for j in range(2 * tb):
                            ps = es_pool.tile([128, 512], bf16, tag="esum",
                                              name="esum")
                            nc.gpsimd.tensor_add(ps[:], e_ap(2 * j),
                                                 e_ap(2 * j + 1))
                            quads.append(ps)
                        # -- interleave previous t-block's output projection --
                        if uT_prev is not None:
                            out_group(4 * prev_tb + h, uT_prev)
                        # -- A @ V (diagonals first so the stop is full) --
                        up_pool = ops_pool if tb == 0 else ups_pool
                        up = up_pool.tile([128, 512], f32,
                                          tag="op" if tb == 0 else "up",
                                          name="up")
                        order = list(range(4 * tb, nsi)) + list(range(4 * tb))
                        for i, si in enumerate(order):
                            v = si - 4 * tb
                            off = DIAG_OFF[v] if v >= 0 else 0
                            g, sv = si // 4, si % 4
                            nc.tensor.matmul(
                                up[:, off:],
                                vG[g][:, 512 * sv + 128 * h:
                                      512 * sv + 128 * (h + 1)],
                                e_ap(si, off), start=(i == 0),
                                stop=(i == nsi - 1))
                        # -- denominator reduce (pairs first, diags last) --
                        den = dps_pool.tile([128, 512], f32, tag="den",
                                            name="den")
                        for i, qs in enumerate(quads):
                            nc.tensor.matmul(den[:], ones[:], qs[:],
                                             start=(i == 0), stop=False)
                        for v in range(4):
                            off = DIAG_OFF[v]
                            nc.tensor.matmul(den[:, off:], ones[:],
                                             e_ap(4 * tb + v, off),
                                             start=(not quads and v == 0),
                                             stop=(v == 3))
                        rec = rcp_pool.tile([128, 512], f32, tag="rec",
                                            name="rec")
                        nc.vector.reciprocal_approx_fast(out=rec[:],
                                                         in_=den[:])
                        uTt = ut_pool.tile([128, 512], bf16, tag=f"uT{h}",
                                           name=f"uT{h}")
                        nc.vector.tensor_mul(uTt[:], up[:], rec[:])
                        uT_cur.append(uTt)
                    uT_prev = uT_cur
                    prev_tb = tb
                # -- final t-block's output projection --
                for ttl in range(4):
                    out_group(4 * prev_tb + ttl, uT_prev, final=True)

    nc.compile()
    return nc


def _get_module():
    if "nc" not in _NC_CACHE:
        _NC_CACHE["nc"] = _build_module()
    return _NC_CACHE["nc"]


def _tile_x(xt):
    """[E, S] -> [8, 4, 128, 2, 512] tile layout (p = e % 128 within 256-slab)."""
    t = xt.reshape(8, 2, 128, 4, 512).transpose(0, 3, 2, 1, 4)
    return np.ascontiguousarray(t).astype(_BF16)


def _host_prep(inputs_q, inputs_kv, positions, Wq, Wk, Wv, Wo):
    """Build the 8 per-core input maps."""
    perm = np.concatenate([np.arange(0, D, 2), np.arange(1, D, 2)])  # de-interleave
    scale = np.float32(1.0 / np.sqrt(D))
    half = D // 2
    timescale = 10000.0 ** (2.0 * np.arange(half, dtype=np.float64) / D)
    ones = np.ones((128, 128), dtype=_BF16)
    # shifted causal mask table: M[s, c] = MASK iff (c - 128) < s
    s_i = np.arange(128)[:, None]
    c_i = np.arange(640)[None, :]
    msk = np.where(c_i - 128 < s_i, MASK_VALUE, 0.0).astype(np.float32)

    in_maps = []
    for c in range(8):
        b = c // 4
        h0 = (c % 4) * HL
        angle = positions[b].astype(np.float64)[None, :] / timescale[:, None]  # [64,S]
        cs = np.cos(angle).astype(np.float32)
        sn = np.sin(angle).astype(np.float32)
        csd = np.concatenate([cs, cs], axis=0)               # [128, S]
        sns = np.concatenate([-sn, sn], axis=0)              # [128, S]
        wq = (Wq[:, h0:h0 + HL, :][:, :, perm] * scale).reshape(E, ND)
        wk = Wk[:, h0:h0 + HL, :][:, :, perm].reshape(E, ND)
        wv = Wv[:, h0:h0 + HL, :].reshape(E, ND)
        wo = Wo[h0:h0 + HL].reshape(ND, E)
        in_maps.append({
            "xq_t": _tile_x(inputs_q[b].T),
            "xkv_t": _tile_x(inputs_kv[b].T),
            "wq": np.ascontiguousarray(wq).astype(_BF16),
            "wk": np.ascontiguousarray(wk).astype(_BF16),
            "wv": np.ascontiguousarray(wv).astype(_BF16),
            "wo": np.ascontiguousarray(wo).astype(_BF16),
            "csd": csd.astype(_BF16), "sns": sns.astype(_BF16), "ones": ones, "msk": msk,
        })
    return in_maps


def kernel(inputs_q, inputs_kv, positions, Wq, Wk, Wv, Wo, _trace=False,
           _trace_kwargs=None):
    from concourse import bass_utils

    nc = _get_module()
    in_maps = _host_prep(inputs_q, inputs_kv, positions, Wq, Wk, Wv, Wo)
    res = bass_utils.run_bass_kernel_spmd(
        nc, in_maps, core_ids=list(range(8)), trace=_trace,
        **(_trace_kwargs or {}))
    if _trace:
        _NC_CACHE["last_results"] = res
    parts = [res.results[c]["out"] for c in range(8)]
    out0 = parts[0] + parts[1] + parts[2] + parts[3]
    out1 = parts[4] + parts[5] + parts[6] + parts[7]
    return np.stack([out0, out1]).astype(np.float32)


# revision 29
# speedup vs baseline: 1.0426x; 1.0426x over previous
"""Multi-head dot-product attention (RoPE, causal) on 8 NeuronCores.

Sharding: data-parallel over batch (2) x tensor-parallel over heads (16 -> 4
per core). Each core projects q/k/v for its 4 heads, runs causal attention,
and computes a partial output projection; the host sums the 4 partials per
batch element.

Device layout notes:
- All matmul operands are float32r (full-rate on the PE for moving dims
  >= 256; matmuls are emitted at width >= 256).
- Inputs are fed pre-transposed ([E, S]) so projections contract E on
  partitions; q/k are produced transposed per head ([D, S]).
- Scores are computed transposed (ST[s, t]) so the A@V contraction needs no
  on-chip transposes; softmax skips max-subtraction (scores are O(1) by
  construction).
- Attention loops t-blocks outer / heads inner; the previous t-block's
  output projection is interleaved between a head's QK and AV matmuls so
  the PE never waits on the exp (Activation engine) pipeline.
- Diagonal score sub-blocks are emitted at narrowed moving widths
  (512/384/256/256) with a single shifted causal mask table; the masked
  prefix of each narrowed e-tile is never read.
- The softmax denominator is accumulated on the Pool engine (esum) and
  reduced across partitions with ONE all-ones matmul per (h, t-block);
  1/den uses the fast DVE reciprocal approximation.
- RoPE uses a de-interleaved head dim (even dims | odd dims), folded into a
  host-side permutation of Wq/Wk columns; scores are permutation-invariant.
"""

import numpy as np
import ml_dtypes

_BF16 = ml_dtypes.bfloat16

B, S, E, N, D = 2, 2048, 2048, 16, 128
HL = 4           # local heads per core (8 cores = 2 batch x 4 head groups)
ND = HL * D      # 512
NT = S // 128    # 16 row tiles
NB = S // 512    # 4 row blocks
NE = E // 128    # 16 contraction tiles
MASK_VALUE = float(-0.7 * np.finfo(np.float32).max)

# diagonal sub-block emission: v -> (column offset, width) within the t-block
DIAG_OFF = (0, 128, 256, 256)
DIAG_W = (512, 384, 256, 256)
# mask table column slices ([128, 640] table, M[s, c] = MASK iff c-128 < s)
DIAG_MSK = ((128, 640), (128, 512), (128, 384), (0, 256))

_NC_CACHE = {}


def _build_module():
    import concourse.bass as bass
    import concourse.mybir as mybir
    import concourse.tile as tile
    from concourse import bacc

    f32 = mybir.dt.float32
    f32r = mybir.dt.float32r
    bf16 = mybir.dt.bfloat16
    Exp = mybir.ActivationFunctionType.Exp

    nc = bacc.Bacc("TRN2", target_bir_lowering=False, debug=False, num_devices=8)

    xq_d = nc.dram_tensor("xq_t", [NE // 2, NB, 128, 2, 512], bf16, kind="ExternalInput").ap()
    xkv_d = nc.dram_tensor("xkv_t", [NE // 2, NB, 128, 2, 512], bf16, kind="ExternalInput").ap()
    wq_d = nc.dram_tensor("wq", [E, ND], bf16, kind="ExternalInput").ap()
    wk_d = nc.dram_tensor("wk", [E, ND], bf16, kind="ExternalInput").ap()
    wv_d = nc.dram_tensor("wv", [E, ND], bf16, kind="ExternalInput").ap()
    wo_d = nc.dram_tensor("wo", [ND, E], bf16, kind="ExternalInput").ap()
    csd_d = nc.dram_tensor("csd", [128, S], bf16, kind="ExternalInput").ap()
    sns_d = nc.dram_tensor("sns", [128, S], bf16, kind="ExternalInput").ap()
    ones_d = nc.dram_tensor("ones", [128, 128], bf16, kind="ExternalInput").ap()
    msk_d = nc.dram_tensor("msk", [128, 640], f32, kind="ExternalInput").ap()
    out_d = nc.dram_tensor("out", [S, E], f32, kind="ExternalOutput").ap()

    def load_w_tiles(pool, dram, tag, ets, engine=None):
        """Per-et weight tiles: [128, 512] bf16 slabs of [E, ND]."""
        ws = []
        for et in ets:
            w = pool.tile([128, ND], bf16, tag=f"{tag}{et}", name=f"{tag}{et}")
            (engine or nc.gpsimd).dma_start(
                w[:], dram[bass.ds(128 * et, 128), :])
            ws.append(w)
        return ws

    with tile.TileContext(nc) as tc:
        with tc.tile_pool(name="qkp", bufs=1) as qk_pool, \
             tc.tile_pool(name="vp", bufs=1) as v_pool, \
             tc.tile_pool(name="cst", bufs=1) as cpool:
            qT = [qk_pool.tile([128, S], bf16, tag=f"qT{h}", name=f"qT{h}")
                  for h in range(HL)]
            kT = [qk_pool.tile([128, S], bf16, tag=f"kT{h}", name=f"kT{h}")
                  for h in range(HL)]
            vG = [v_pool.tile([128, 4 * ND], bf16, tag=f"vG{g}",
                              name=f"vG{g}") for g in range(4)]
            ones = cpool.tile([128, 128], bf16, tag="ones")
            msk = cpool.tile([128, 640], f32, tag="msk")

            # ---- fused QKV projection (RoPE tables live only here) ----
            with nc.named_scope("proj"), \
                 tc.tile_pool(name="tables", bufs=1) as tpool, \
                 tc.tile_pool(name="wqp", bufs=1) as wq_pool, \
                 tc.tile_pool(name="wkp", bufs=1) as wk_pool, \
                 tc.tile_pool(name="wvp", bufs=1) as wv_pool, \
                 tc.tile_pool(name="xp", bufs=16) as xpool, \
                 tc.tile_pool(name="qps", bufs=2, space="PSUM") as qps_pool, \
                 tc.tile_pool(name="rope", bufs=2) as rope_pool:
                csd = tpool.tile([128, S], bf16, tag="csd")
                sns = tpool.tile([128, S], bf16, tag="sns")

                def rope(dst, src_ps, tb):
                    tbs = bass.ts(tb, 512)
                    tmp = rope_pool.tile([128, 512], f32, tag="tmp",
                                         name="tmp")
                    nc.vector.tensor_mul(tmp[0:64, :], src_ps[64:128, :],
                                         sns[0:64, tbs])
                    nc.vector.tensor_mul(tmp[64:128, :], src_ps[0:64, :],
                                         sns[64:128, tbs])
                    nc.vector.tensor_mul(dst[:, tbs], src_ps[:],
                                         csd[:, tbs])
                    nc.vector.tensor_add(dst[:, tbs], dst[:, tbs], tmp[:])

                wq = load_w_tiles(wq_pool, wq_d, "wq", range(NE))
                wk = load_w_tiles(wk_pool, wk_d, "wk", range(NE))
                wv = load_w_tiles(wv_pool, wv_d, "wv", range(4))

                def x_tile(dram, engine, tag, ep, tb, split=False):
                    x = xpool.tile([128, 2, 512], bf16, tag=tag,
                                   name=tag)
                    srcv = dram[ep, tb]
                    if split:
                        engine.dma_start(x[:, 0], srcv[:, 0])
                        engine.dma_start(x[:, 1], srcv[:, 1])
                    else:
                        engine.dma_start(x[:], srcv)
                    return x

                for tb in range(NB):
                    # xq: 8 tiles across 2 rotating tags (bufs=10 -> deep)
                    xq = [x_tile(xq_d,
                                 nc.sync if ep % 2 == 0 else nc.scalar,
                                 "xq", ep, tb,
                                 split=(tb == 0 and ep == 0))
                          for ep in range(NE // 2)]
                    xkv = [x_tile(xkv_d,
                                  nc.scalar if ep % 2 == 0 else nc.sync,
                                  "xkv", ep, tb)
                           for ep in range(NE // 2)]
                    if tb == 0:
                        nc.sync.dma_start(csd[:], csd_d[:])
                        nc.sync.dma_start(sns[:], sns_d[:])
                        nc.scalar.dma_start(ones[:], ones_d[:])
                        nc.scalar.dma_start(msk[:], msk_d[:])
                        wv += load_w_tiles(wv_pool, wv_d, "wv",
                                           range(4, NE), engine=nc.sync)
                    # -- Q subloop --
                    qps = [qps_pool.tile([128, 512], f32, tag=f"q{h}",
                                         name=f"qps{h}") for h in range(HL)]
                    for et in range(NE):
                        for h in range(HL):
                            nc.tensor.matmul(
                                qps[h][:], wq[et][:, bass.ts(h, 128)],
                                xq[et // 2][:, et % 2], start=(et == 0),
                                stop=(et == NE - 1))
                    for h in range(HL):
                        rope(qT[h], qps[h][:], tb)
                    # -- K subloop --
                    kps = [qps_pool.tile([128, 512], f32, tag=f"q{h}",
                                         name=f"kps{h}") for h in range(HL)]
                    for et in range(NE):
                        for h in range(HL):
                            nc.tensor.matmul(
                                kps[h][:], wk[et][:, bass.ts(h, 128)],
                                xkv[et // 2][:, et % 2], start=(et == 0),
                                stop=(et == NE - 1))
                    for h in range(HL):
                        rope(kT[h], kps[h][:], tb)
                    # -- V subloop --
                    for sv in range(4):
                        vps = qps_pool.tile([128, ND], f32, tag=f"q{sv}",
                                            name=f"vps{sv}")
                        for et in range(NE):
                            nc.tensor.matmul(
                                vps[:], xkv[et // 2][:, et % 2, bass.ts(sv, 128)],
                                wv[et][:], start=(et == 0),
                                stop=(et == NE - 1))
                        nc.scalar.copy(vG[tb][:, bass.ts(sv, 512)], vps[:])

            # ---- Attention + output projection (interleaved) ----
            with nc.named_scope("attn"), \
                 tc.tile_pool(name="wop", bufs=1) as wo_pool, \
                 tc.tile_pool(name="et", bufs=1) as et_pool, \
                 tc.tile_pool(name="uTp", bufs=2) as ut_pool, \
                 tc.tile_pool(name="esp", bufs=12) as es_pool, \
                 tc.tile_pool(name="rcp", bufs=2) as rcp_pool, \
                 tc.tile_pool(name="ob", bufs=3) as ob_pool, \
                 tc.tile_pool(name="sps", bufs=2, space="PSUM") as sps_pool, \
                 tc.tile_pool(name="ups", bufs=1, space="PSUM") as ups_pool, \
                 tc.tile_pool(name="dps", bufs=1, space="PSUM") as dps_pool, \
                 tc.tile_pool(name="ops", bufs=2, space="PSUM") as ops_pool:
                wo = []
                for h in range(HL):
                    w = wo_pool.tile([128, E], bf16, tag=f"wo{h}",
                                     name=f"wo{h}")
                    nc.sync.dma_start(w[:], wo_d[bass.ts(h, 128), :])
                    wo.append(w)
                eG = [et_pool.tile([128, 2048], bf16, tag=f"eG{g}",
                                   name=f"eG{g}") for g in range(4)]

                def e_ap(si, lo=0, hi=512):
                    base = 512 * (si % 4)
                    return eG[si // 4][:, base + lo:base + hi]

                def out_group(tt, uTt, final=False):
                    """Output projection for one 128-row t-tile."""
                    ttl = tt % 4
                    for ec in range(4):
                        op = ops_pool.tile([128, 512], f32, tag="op",
                                           name="op")
                        for hh in range(HL):
                            nc.tensor.matmul(
                                op[:], uTt[hh][:, bass.ts(ttl, 128)],
                                wo[hh][:, bass.ts(ec, 512)],
                                start=(hh == 0), stop=(hh == HL - 1))
                        ob = ob_pool.tile([128, 512], f32, tag="ob",
                                          name="ob")
                        nc.vector.tensor_copy(out=ob[:], in_=op[:])
                        eng = nc.scalar if (final and ec % 2) else nc.sync
                        eng.dma_start(
                            out_d[bass.ds(128 * tt, 128),
                                  bass.ds(512 * ec, 512)], ob[:])

                uT_prev = None
                prev_tb = None
                for tb in range(NB):
                    tbs = bass.ts(tb, 512)
                    nsi = 4 * (tb + 1)
                    uT_cur = []
                    if tb == 0:
                        # tb0: no out-projection work exists to cover the
                        # exp latency, so emit ALL heads' QK/mask/exp first
                        # (each head gets a private eG group), then AV/den.
                        for h in range(HL):
                            spd = [sps_pool.tile([128, 2, 512], f32,
                                                 tag="sp", name="sp")
                                   for _ in range(2)]
                            for v in range(4):
                                off = DIAG_OFF[v]
                                sl = spd[v // 2][:, v % 2, off:512]
                                nc.tensor.matmul(
                                    sl, kT[h][:, bass.ts(v, 128)],
                                    qT[h][:, off:512], start=True, stop=True)
                                m0, m1 = DIAG_MSK[v]
                                nc.vector.tensor_add(sl, sl, msk[:, m0:m1])
                                nc.scalar.activation(
                                    eG[h][:, 512 * v + off:512 * (v + 1)],
                                    sl, Exp)
                        for h in range(HL):
                            up_pool = ops_pool
                            up = up_pool.tile([128, 512], f32, tag="op",
                                              name="up")
                            for i, v in enumerate(range(4)):
                                off = DIAG_OFF[v]
                                nc.tensor.matmul(
                                    up[:, off:],
                                    vG[0][:, 512 * v + 128 * h:
                                          512 * v + 128 * (h + 1)],
                                    eG[h][:, 512 * v + off:512 * (v + 1)],
                                    start=(v == 0), stop=(v == 3))
                            den = dps_pool.tile([128, 512], f32, tag="den",
                                                name="den")
                            for v in range(4):
                                off = DIAG_OFF[v]
                                nc.tensor.matmul(
                                    den[:, off:], ones[:],
                                    eG[h][:, 512 * v + off:512 * (v + 1)],
                                    start=(v == 0), stop=(v == 3))
                            rec = rcp_pool.tile([128, 512], f32, tag="rec",
                                                name="rec")
                            nc.vector.reciprocal_approx_fast(out=rec[:],
                                                             in_=den[:])
                            uTt = ut_pool.tile([128, 512], bf16,
                                               tag=f"uT{h}", name=f"uT{h}")
                            nc.vector.tensor_mul(uTt[:], up[:], rec[:])
                            uT_cur.append(uTt)
                        uT_prev = uT_cur
                        prev_tb = tb
                        continue
                    for h in range(HL):
                        # -- QK: full-width si pairs below the diagonal --
                        for j in range(2 * tb):
                            sp = sps_pool.tile([128, 2, 512], f32,
                                               tag="sp", name="sp")
                            for p2 in range(2):
                                si = 2 * j + p2
                                nc.tensor.matmul(
                                    sp[:, p2], kT[h][:, bass.ts(si, 128)],
                                    qT[h][:, tbs], start=True, stop=True)
                            nc.scalar.activation(
                                e_ap(2 * j, 0, 1024),
                                sp[:].rearrange("p a b -> p (a b)"), Exp)
                        # -- QK: narrowed diagonal sub-blocks --
                        spd = [sps_pool.tile([128, 2, 512], f32, tag="sp",
                                             name="sp") for _ in range(2)]
                        for v in range(4):
                            si = 4 * tb + v
                            off, w = DIAG_OFF[v], DIAG_W[v]
                            sl = spd[v // 2][:, v % 2, off:512]
                            nc.tensor.matmul(
                                sl, kT[h][:, bass.ts(si, 128)],
                                qT[h][:, 512 * tb + off:512 * (tb + 1)],
                                start=True, stop=True)
                            m0, m1 = DIAG_MSK[v]
                            nc.vector.tensor_add(sl, sl, msk[:, m0:m1])
                            nc.scalar.activation(e_ap(si, off), sl, Exp)
                        # -- Pool pair/quad sums of e (non-diag) --
                        quads = []
                        for j in range(2 * tb):
                            ps = es_pool.tile([128, 512], bf16, tag="esum",
                                              name="esum")
                            nc.gpsimd.tensor_add(ps[:], e_ap(2 * j),
                                                 e_ap(2 * j + 1))
                            quads.append(ps)
                        # -- interleave previous t-block's output projection --
                        if uT_prev is not None:
                            out_group(4 * prev_tb + h, uT_prev)
                        # -- A @ V (diagonals first so the stop is full) --
                        up_pool = ops_pool if tb == 0 else ups_pool
                        up = up_pool.tile([128, 512], f32,
                                          tag="op" if tb == 0 else "up",
                                          name="up")
                        order = list(range(4 * tb, nsi)) + list(range(4 * tb))
                        for i, si in enumerate(order):
                            v = si - 4 * tb
                            off = DIAG_OFF[v] if v >= 0 else 0
                            g, sv = si // 4, si % 4
                            nc.tensor.matmul(
                                up[:, off:],
                                vG[g][:, 512 * sv + 128 * h:
                                      512 * sv + 128 * (h + 1)],
                                e_ap(si, off), start=(i == 0),
                                stop=(i == nsi - 1))
                        # -- denominator reduce (pairs first, diags last) --
                        den = dps_pool.tile([128, 512], f32, tag="den",
                                            name="den")
                        for i, qs in enumerate(quads):
                            nc.tensor.matmul(den[:], ones[:], qs[:],
                                             start=(i == 0), stop=False)
                        for v in range(4):
                            off = DIAG_OFF[v]
                            nc.tensor.matmul(den[:, off:], ones[:],
                                             e_ap(4 * tb + v, off),
                                             start=(not quads and v == 0),
                                             stop=(v == 3))
                        rec = rcp_pool.tile([128, 512], f32, tag="rec",
                                            name="rec")
                        nc.vector.reciprocal_approx_fast(out=rec[:],
                                                         in_=den[:])
                        uTt = ut_pool.tile([128, 512], bf16, tag=f"uT{h}",
                                           name=f"uT{h}")
                        nc.vector.tensor_mul(uTt[:], up[:], rec[:])
                        uT_cur.append(uTt)
                    uT_prev = uT_cur
                    prev_tb = tb
                # -- final t-block's output projection --
                for ttl in range(4):
                    out_group(4 * prev_tb + ttl, uT_prev, final=True)

    nc.compile()
    return nc


def _get_module():
    if "nc" not in _NC_CACHE:
        _NC_CACHE["nc"] = _build_module()
    return _NC_CACHE["nc"]


def _tile_x(xt):
    """[E, S] -> [8, 4, 128, 2, 512] tile layout (p = e % 128 within 256-slab)."""
    t = xt.reshape(8, 2, 128, 4, 512).transpose(0, 3, 2, 1, 4)
    return np.ascontiguousarray(t).astype(_BF16)


def _host_prep(inputs_q, inputs_kv, positions, Wq, Wk, Wv, Wo):
    """Build the 8 per-core input maps."""
    perm = np.concatenate([np.arange(0, D, 2), np.arange(1, D, 2)])  # de-interleave
    scale = np.float32(1.0 / np.sqrt(D))
    half = D // 2
    timescale = 10000.0 ** (2.0 * np.arange(half, dtype=np.float64) / D)
    ones = np.ones((128, 128), dtype=_BF16)
    # shifted causal mask table: M[s, c] = MASK iff (c - 128) < s
    s_i = np.arange(128)[:, None]
    c_i = np.arange(640)[None, :]
    msk = np.where(c_i - 128 < s_i, MASK_VALUE, 0.0).astype(np.float32)

    in_maps = []
    for c in range(8):
        b = c // 4
        h0 = (c % 4) * HL
        angle = positions[b].astype(np.float64)[None, :] / timescale[:, None]  # [64,S]
        cs = np.cos(angle).astype(np.float32)
        sn = np.sin(angle).astype(np.float32)
        csd = np.concatenate([cs, cs], axis=0)               # [128, S]
        sns = np.concatenate([-sn, sn], axis=0)              # [128, S]
        wq = (Wq[:, h0:h0 + HL, :][:, :, perm] * scale).reshape(E, ND)
        wk = Wk[:, h0:h0 + HL, :][:, :, perm].reshape(E, ND)
        wv = Wv[:, h0:h0 + HL, :].reshape(E, ND)
        wo = Wo[h0:h0 + HL].reshape(ND, E)
        in_maps.append({
            "xq_t": _tile_x(inputs_q[b].T),
            "xkv_t": _tile_x(inputs_kv[b].T),
            "wq": np.ascontiguousarray(wq).astype(_BF16),
            "wk": np.ascontiguousarray(wk).astype(_BF16),
            "wv": np.ascontiguousarray(wv).astype(_BF16),
            "wo": np.ascontiguousarray(wo).astype(_BF16),
            "csd": csd.astype(_BF16), "sns": sns.astype(_BF16), "ones": ones, "msk": msk,
        })
    return in_maps


def kernel(inputs_q, inputs_kv, positions, Wq, Wk, Wv, Wo, _trace=False,
           _trace_kwargs=None):
    from concourse import bass_utils

    nc = _get_module()
    in_maps = _host_prep(inputs_q, inputs_kv, positions, Wq, Wk, Wv, Wo)
    res = bass_utils.run_bass_kernel_spmd(
        nc, in_maps, core_ids=list(range(8)), trace=_trace,
        **(_trace_kwargs or {}))
    if _trace:
        _NC_CACHE["last_results"] = res
    parts = [res.results[c]["out"] for c in range(8)]
    out0 = parts[0] + parts[1] + parts[2] + parts[3]
    out1 = parts[4] + parts[5] + parts[6] + parts[7]
    return np.stack([out0, out1]).astype(np.float32)
